# revision 68
# baseline (speedup 1.0000x reference)
"""GPTBigCode MQA attention layer on 8 TRN2 NeuronCores.

Sharding: data-parallel over batch (2) x tensor-parallel over head groups
(4 groups of 4 heads).  Core = (b, g).  Each core computes:
  qkvT = W_qkv[:, cols].T @ X[b].T        (cols = Q cols of group + shared KV)
  per head: scoresT = K^T.T @ Q^T, P = exp(scale*scoresT) (causal, no max-sub),
  attnT = V.T @ P, col-sums via ones-matmul, scale by 1/sum,
  partialT = W_proj[rows].T @ attnT
Host gathers: out[b] = sum_g partialT.T + c_proj_b.

Schedule notes (final, ~215us/core design model vs 309us baseline):
  - PE executes in program order, so the whole kernel is emitted as a
    software pipeline: per head, all scores matmuls + exps go first and
    the PV burst runs one head behind; the last head of each macro is
    deferred across the next macro's QKV window; c_proj for macro m is
    interleaved into macro m+1's attention bursts.
  - scores land in f32 [128,2,512] PSUM pair tiles (two k-tiles per
    bank): deepens the PE lookahead the PSUM ring allows and halves the
    exp instruction count for full tiles.
  - causal restriction: diagonal k-tiles compute only q >= j*128 within
    the macro; one [128,128] triangle mask handles the partial block.
  - softmax denominators entirely off the PE: DVE accumulates exp tiles
    in fp16 (2x mode), one closing ones-matmul per head.
  - QKV runs t-outer with 3 concurrent chains (2 mmA banks + 1 scp
    half-bank) tracking the startup DMA stream; W_qkv host-permuted to
    [K, V, Q0..3] so pass A needs only cols 0:384.
  - V^T -> V transpose on the DMA xbar (dma_start_transpose), no PE.
  - input DMAs paired via 3D APs (each DMA costs ~630ns on the shared
    HWDGE queue); xt prefetched for all macros; outputs staged bf16
    [128,4,512] per group, final group per-eo to shorten the drain, and
    c_proj group 0 of the last macro pre-opens heads 0..2 to bridge the
    last head's normalize latency.

All matmuls bf16 with fp32 PSUM accumulation.
"""

import sys

sys.path.insert(0, "/opt/trn_rl_repo")

import numpy as np
import ml_dtypes

S = 2048
E = 2048
D = 128
HG = 4  # heads per core
SM = 4  # s macro-tiles of 512
ET = 16  # e (contraction) tiles of 128
SCALE = 1.0 / float(np.sqrt(D))  # net softmax input scale (unscale cancels)

_CACHE = {}


def _build_nc(with_bias=False):
    import concourse.bacc as bacc
    import concourse.mybir as mybir
    import concourse.tile as tile
    from concourse.bass import ds, ts
    from contextlib import ExitStack

    BF = mybir.dt.bfloat16
    F32 = mybir.dt.float32
    F16 = mybir.dt.float16
    Act = mybir.ActivationFunctionType

    nc = bacc.Bacc("TRN2", target_bir_lowering=False, debug=False)

    xt_d = nc.dram_tensor("xt", (E, S), BF, kind="ExternalInput")
    wq_d = nc.dram_tensor("w_qkv", (E, 768), BF, kind="ExternalInput")
    bq_d = (
        nc.dram_tensor("b_qkv", (768, 1), F32, kind="ExternalInput")
        if with_bias
        else None
    )
    wp_d = nc.dram_tensor("w_proj", (512, E), BF, kind="ExternalInput")
    mk_d = nc.dram_tensor("maskt", (128, 128), BF, kind="ExternalInput")
    out_d = nc.dram_tensor("outT", (E, S), BF, kind="ExternalOutput")

    with ExitStack() as ctx:
        tc = ctx.enter_context(tile.TileContext(nc))
        const = ctx.enter_context(tc.tile_pool(name="const", bufs=1))
        xpool = ctx.enter_context(tc.tile_pool(name="xpool", bufs=2))
        ptpool = ctx.enter_context(tc.tile_pool(name="ptpool", bufs=28))
        rpool = ctx.enter_context(tc.tile_pool(name="rpool", bufs=2))
        spool = ctx.enter_context(tc.tile_pool(name="spool", bufs=2))
        opool = ctx.enter_context(tc.tile_pool(name="opool", bufs=3))
        psA = ctx.enter_context(tc.tile_pool(name="psA", bufs=2, space="PSUM"))
        scp = ctx.enter_context(tc.tile_pool(name="scp", bufs=2, space="PSUM"))
        psC = ctx.enter_context(tc.tile_pool(name="psC", bufs=2, space="PSUM"))

        # --- persistent SBUF tensors ---
        wq_sb = const.tile([128, ET, 768], BF, tag="wq")  # W_qkv, e-tile major
        wp_sb = const.tile([128, 4, E], BF, tag="wp")  # W_proj, c-tile major
        mk_sb = const.tile([128, 128], BF, tag="mk")  # causal triangle 0/1
        bq_sb = const.tile([128, 6], F32, tag="bq")  # qkv bias per c-tile
        qk_sb = const.tile([128, 5, S], BF, tag="qk")  # Q^T (4 heads) + K^T
        vt_sb = const.tile([128, S], BF, tag="vt")  # V^T staging
        v_sb = const.tile([128, ET, D], BF, tag="v")  # V, k-tile major
        at_sb = const.tile([128, HG, S], BF, tag="at")  # scaled attnT per head
        ones_h = const.tile([128, 1], F16, tag="ones_h")

        # --- DMAs.  W_qkv columns are host-permuted to [K, V, Q0..Q3] so
        # pass A (K, V, Q0) needs only wq cols 0:384.  Small consts go on
        # the Act queue; SP streams paired (xt, wq) tiles for fast startup.
        if with_bias:
            for c in range(6):
                nc.scalar.dma_start(
                    out=bq_sb[:, c : c + 1], in_=bq_d[ts(c, 128), :]
                )
        nc.scalar.dma_start(
            out=wq_sb[:, 0, ds(0, 384)], in_=wq_d[ts(0, 128), ds(0, 384)]
        )
        nc.scalar.dma_start(out=mk_sb, in_=mk_d[:, :])
        nc.vector.memset(ones_h, 1.0)

        xts = []
        for m in range(SM):
            xts.append(xpool.tile([128, ET, 512], BF, tag="xt", name=f"xts{m}"))

        def _dram_pair(dram, t0, c0, w):
            return dram[ds(t0 * 128, 256), ds(c0, w)].rearrange(
                "(t p) c -> p t c", p=128
            )

        def _dram_grp(dram, t0, n, c0, w):
            return dram[ds(t0 * 128, n * 128), ds(c0, w)].rearrange(
                "(t p) c -> p t c", p=128
            )

        # graduated first wave: singles, then pairs, then quads — small
        # tiles give the PE an early start, larger ones amortize the
        # ~630ns HWDGE issue cost once the pipeline is rolling.
        nc.sync.dma_start(out=xts[0][:, 0, :], in_=xt_d[ts(0, 128), ds(0, 512)])
        nc.sync.dma_start(out=xts[0][:, 1, :], in_=xt_d[ts(1, 128), ds(0, 512)])
        nc.sync.dma_start(
            out=wq_sb[:, 1, ds(0, 384)], in_=wq_d[ts(1, 128), ds(0, 384)]
        )
        for t0, n in ((2, 2), (4, 2), (6, 2), (8, 2), (10, 2), (12, 2), (14, 2)):
            nc.sync.dma_start(
                out=xts[0][:, ds(t0, n), :], in_=_dram_grp(xt_d, t0, n, 0, 512)
            )
            nc.sync.dma_start(
                out=wq_sb[:, ds(t0, n), ds(0, 384)],
                in_=_dram_grp(wq_d, t0, n, 0, 384),
            )
        for i in range(8):
            nc.sync.dma_start(
                out=wq_sb[:, ds(2 * i, 2), ds(384, 384)],
                in_=_dram_pair(wq_d, 2 * i, 384, 384),
            )
        # prefetch xt for later macros (ring-buffered); wp after xt/wq
        for m in range(1, SM):
            for i in range(8):
                nc.sync.dma_start(
                    out=xts[m][:, ds(2 * i, 2), :],
                    in_=_dram_pair(xt_d, 2 * i, m * 512, 512),
                )
        for c in range(4):
            nc.sync.dma_start(out=wp_sb[:, c, :], in_=wp_d[ts(c, 128), :])

        def _qkv(m):
            # QKV projection, t-outer with 3 concurrent chains.
            # c-block order (host-permuted): 0=K, 1=V, 2..5=Q heads 0..3.
            sm = ds(m * 512, 512)

            def _qkv_dest(c):
                if c == 0:
                    return qk_sb[:, 4, sm]
                if c == 1:
                    return vt_sb[:, sm]
                return qk_sb[:, c - 2, sm]

            for half in range(2):
                cs = [3 * half + i for i in range(3)]
                chains = [
                    psA.tile([128, 512], F32, tag="mmA", name=f"q{m}{c}")
                    for c in cs[:2]
                ]
                chains.append(
                    scp.tile([128, 2, 512], F32, tag="sc", name=f"q{m}x")[:, 0, :]
                )
                for t in range(ET):
                    for ci, c in enumerate(cs):
                        nc.tensor.matmul(
                            chains[ci],
                            lhsT=wq_sb[:, t, ds(c * 128, 128)],
                            rhs=xts[m][:, t, :],
                            start=(t == 0),
                            stop=(t == ET - 1),
                        )
                for ci, c in enumerate(cs):
                    if with_bias:
                        nc.vector.tensor_scalar_add(
                            _qkv_dest(c), chains[ci], bq_sb[:, c : c + 1]
                        )
                    else:
                        nc.vector.tensor_copy(
                            out=_qkv_dest(c), in_=chains[ci]
                        )
                if half == 0:
                    # V^T ready after pass 0 (c-blocks K, V): transpose the
                    # whole macro slice on the DMA xbar, no PE/PSUM needed.
                    nc.sync.dma_start_transpose(
                        out=v_sb[:, ds(4 * m, 4), :], in_=vt_sb[:, sm]
                    )

        def _attention(m, cproj_m=None, pend=None):
            sm = ds(m * 512, 512)
            # Diagonal k-tiles only touch q >= j*128 within the macro, so
            # scores/exp/PV run on restricted column ranges.  The softmax
            # denominator is split: the first `npe` (full-width) tiles are
            # summed by PE ones-matmuls; the tail is accumulated on DVE in
            # fp32 and closed out with one final ones-matmul.
            nkt = 4 * (m + 1)
            npe = [1, 3, 2, 4][m]

            def _scores(h):
                # scores land in fp16 PSUM pair tiles (two k-tiles share a
                # bank): doubles the PE lookahead the PSUM ring allows and
                # halves the exp instruction count for full tiles.
                pts = []
                kt = 0
                while kt < nkt:
                    j = kt - 4 * m
                    if j < 0 and kt + 1 < 4 * m:
                        sc = scp.tile([128, 2, 512], F32, tag="sc")
                        ptp = ptpool.tile(
                            [128, 2, 512], BF, tag="pt", name=f"p{m}{h}"
                        )
                        for i in (0, 1):
                            nc.tensor.matmul(
                                sc[:, i, :],
                                lhsT=qk_sb[:, 4, ds((kt + i) * 128, 128)],
                                rhs=qk_sb[:, h, ds(m * 512, 512)],
                                start=True,
                                stop=True,
                            )
                        nc.scalar.activation(
                            out=ptp, in_=sc, func=Act.Exp, bias=0.0, scale=SCALE
                        )
                        pts.append((ptp[:, 0, :], 0, 512))
                        pts.append((ptp[:, 1, :], 0, 512))
                        kt += 2
                    else:
                        off = j * 128 if j > 0 else 0
                        w = 512 - off
                        sc = scp.tile([128, 2, 512], F32, tag="sc")
                        ptp = ptpool.tile(
                            [128, 2, 512], BF, tag="pt", name=f"p{m}{h}"
                        )
                        nc.tensor.matmul(
                            sc[:, 0, ds(0, w)],
                            lhsT=qk_sb[:, 4, ds(kt * 128, 128)],
                            rhs=qk_sb[:, h, ds(m * 512 + off, w)],
                            start=True,
                            stop=True,
                        )
                        nc.scalar.activation(
                            out=ptp[:, 0, ds(off, w)],
                            in_=sc[:, 0, ds(0, w)],
                            func=Act.Exp,
                            bias=0.0,
                            scale=SCALE,
                        )
                        if j >= 0:
                            nc.vector.tensor_mul(
                                ptp[:, 0, ds(off, 128)],
                                ptp[:, 0, ds(off, 128)],
                                mk_sb,
                            )
                        pts.append((ptp[:, 0, :], off, w))
                        kt += 1
                return pts

            def _sums(h, pts):
                # DVE accumulates the tail tiles in fp16; the first two are
                # fused into one add (seed tile is always full width).
                sp = spool.tile([128, 512], F16, tag="spacc")
                pt0, off0, w0 = pts[0]
                pt1, off1, w1 = pts[1]
                if off1:
                    nc.vector.tensor_copy(
                        out=sp[:, ds(0, off1)], in_=pt0[:, ds(0, off1)]
                    )
                nc.vector.tensor_add(
                    sp[:, ds(off1, w1)], pt0[:, ds(off1, w1)], pt1[:, ds(off1, w1)]
                )
                for i in range(2, nkt):
                    pt, off, w = pts[i]
                    nc.vector.tensor_add(
                        sp[:, ds(off, w)], sp[:, ds(off, w)], pt[:, ds(off, w)]
                    )
                return sp

            def _pv(h, pts, sp):
                ps_at = psC.tile([128, 512], F32, tag="attnacc", bufs=1)
                ps_sum = psC.tile([1, 512], F32, tag="sum", bufs=1)
                for kt in range(nkt):
                    pt, off, w = pts[kt]
                    nc.tensor.matmul(
                        ps_at[:, ds(off, w)],
                        lhsT=v_sb[:, kt, :],
                        rhs=pt[:, ds(off, w)],
                        start=(kt == 0),
                        stop=(kt == nkt - 1),
                    )
                nc.tensor.matmul(
                    ps_sum, lhsT=ones_h, rhs=sp, start=True, stop=True
                )
                recip = rpool.tile([1, 512], F32, tag="recip")
                nc.vector.reciprocal(recip, ps_sum)
                bc_sb = rpool.tile([128, 512], F32, tag="bc")
                nc.gpsimd.partition_broadcast(bc_sb, recip)
                nc.vector.tensor_mul(at_sb[:, h, sm], ps_at, bc_sb)

            prev = pend
            for h in range(HG):
                if prev is not None:
                    ph, pp, psums, ppv = prev
                    sp = psums(ph, pp)
                pts = _scores(h)
                if prev is not None:
                    ppv(ph, pp, sp)
                if cproj_m is not None:
                    _cproj_group(cproj_m, h)
                prev = (h, pts, _sums, _pv)
            return prev

        def _cproj_group(m, g):
            # one c_proj output group (4 eo blocks) for s-macro m
            sm = ds(m * 512, 512)
            ob = opool.tile([128, 4, 512], BF, tag="ob")
            for i in range(4):
                eo = g * 4 + i
                ps_o = psA.tile([128, 512], F32, tag="mmA", name=f"o{m}{eo}")
                for c in range(4):
                    nc.tensor.matmul(
                        ps_o,
                        lhsT=wp_sb[:, c, ds(eo * 128, 128)],
                        rhs=at_sb[:, c, sm],
                        start=(c == 0),
                        stop=(c == 3),
                    )
                if m == SM - 1 and g == 3:
                    # final group: per-eo copies (DVE/Act alternating) and
                    # per-eo DMAs so the tail is one eo deep, not four.
                    if i % 2:
                        nc.scalar.activation(
                            out=ob[:, i, :],
                            in_=ps_o,
                            func=Act.Copy,
                            bias=0.0,
                            scale=1.0,
                        )
                    else:
                        nc.vector.tensor_copy(out=ob[:, i, :], in_=ps_o)
                    q = nc.scalar if i % 2 else nc.sync
                    q.dma_start(
                        out=out_d[ds(eo * 128, 128), sm], in_=ob[:, i, :]
                    )
                elif i % 2:
                    nc.scalar.activation(
                        out=ob[:, i, :],
                        in_=ps_o,
                        func=Act.Copy,
                        bias=0.0,
                        scale=1.0,
                    )
                else:
                    nc.vector.tensor_copy(out=ob[:, i, :], in_=ps_o)
            if not (m == SM - 1 and g == 3):
                q = nc.scalar if (m == SM - 1 and g % 2) else nc.sync
                q.dma_start(
                    out=out_d[ds(g * 512, 512), sm].rearrange(
                        "(i p) c -> p i c", p=128
                    ),
                    in_=ob,
                )

        # software pipeline: c_proj for macro m is interleaved into macro
        # m+1's attention bursts, and the last head of each macro is
        # deferred across the next macro's QKV window, so its exp/sum
        # chains drain while the PE runs QKV.
        _qkv(0)
        pend = _attention(0)
        for m in range(1, SM):
            _qkv(m)
            pend = _attention(m, cproj_m=m - 1, pend=pend)
        # finish the deferred last head, then bridge the normalize-chain
        # latency by pre-opening c_proj group 0 on heads 0..2 (2 mmA slots
        # + 2 scp half-banks); head 3 joins once its at_sb lands.
        h, pts, fs, fp = pend
        fp(h, pts, fs(h, pts))
        sm3 = ds((SM - 1) * 512, 512)
        open_chains = []
        for i in range(4):
            if i < 2:
                ps_o = psA.tile([128, 512], F32, tag="mmA", name=f"fo{i}")
            else:
                ps_o = scp.tile([128, 2, 512], F32, tag="sc", name=f"fo{i}")[
                    :, 0, :
                ]
            for c in range(3):
                nc.tensor.matmul(
                    ps_o,
                    lhsT=wp_sb[:, c, ds(i * 128, 128)],
                    rhs=at_sb[:, c, sm3],
                    start=(c == 0),
                    stop=False,
                )
            open_chains.append(ps_o)
        ob = opool.tile([128, 4, 512], BF, tag="ob")
        for i in range(4):
            nc.tensor.matmul(
                open_chains[i],
                lhsT=wp_sb[:, 3, ds(i * 128, 128)],
                rhs=at_sb[:, 3, sm3],
                start=False,
                stop=True,
            )
            nc.vector.tensor_copy(out=ob[:, i, :], in_=open_chains[i])
        nc.sync.dma_start(
            out=out_d[ds(0, 512), sm3].rearrange("(i p) c -> p i c", p=128),
            in_=ob,
        )
        for g in range(1, 4):
            _cproj_group(SM - 1, g)

    nc.compile()
    return nc


def _get_nc(with_bias=False):
    key = f"nc{with_bias}"
    if key not in _CACHE:
        _CACHE[key] = _build_nc(with_bias)
    return _CACHE[key]


def _host_mask():
    k = np.arange(128)[:, None]
    q = np.arange(128)[None, :]
    return (k <= q).astype(ml_dtypes.bfloat16)


def kernel(**inputs):
    from concourse.bass_utils import run_bass_kernel_spmd

    hidden = np.asarray(inputs["hidden_states"], dtype=np.float32)
    caw = np.asarray(inputs["c_attn_w"], dtype=np.float32)
    cab = np.asarray(inputs["c_attn_b"], dtype=np.float32)
    cpw = np.asarray(inputs["c_proj_w"], dtype=np.float32)
    cpb = np.asarray(inputs["c_proj_b"], dtype=np.float32)

    bf16 = ml_dtypes.bfloat16
    maskb = _host_mask()
    xt_by_batch = [hidden[b].T.astype(bf16) for b in range(2)]
    in_maps = []
    for core in range(8):
        b, g = core % 2, core // 2
        # column order [K, V, Q-group]: pass A of the QKV projection only
        # needs the first 384 columns, shrinking the startup DMA wave.
        cols = np.r_[E : E + D, E + D : E + 2 * D, g * 512 : (g + 1) * 512]
        in_maps.append(
            {
                "xt": xt_by_batch[b],
                "w_qkv": caw[:, cols].astype(bf16),
                "b_qkv": cab[cols].reshape(768, 1).astype(np.float32),
                "w_proj": cpw[g * 512 : (g + 1) * 512, :].astype(bf16),
                "maskt": maskb,
            }
        )

    nc = _get_nc(with_bias=bool(np.any(cab)))
    res = run_bass_kernel_spmd(nc, in_maps, core_ids=list(range(8)))
    out = np.zeros((2, S, E), np.float32)
    for core in range(8):
        b = core % 2
        out[b] += res.results[core]["outT"].T.astype(np.float32)
    out += cpb[None, None, :]
    return out
